# revision 36
# baseline (speedup 1.0000x reference)
"""CVQNN classifier kernel for 8 Trainium2 NeuronCores — v5 (fp8 input).

Math: the whole quantum circuit collapses to a batch-independent affine map
(S, d) on 128-dim phase space.  Per batch row the heavy work is
    m = x @ W2 + d20          (W2 = S[rows, :64].T, shape (64, 20))
    out_k = log1p(m_x[k]^2 + m_p[k]^2 + covc_k)
i.e. a (B,64) @ (64,20) matmul + elementwise tail -> (B,10).  Memory bound.

vs the bf16 baseline (88.3us):
  - x ships as float8 e3m4 (max 15.5; e4m3 would blow the 2e-2 budget,
    e3m4 measures 1.47e-2 end-to-end) -> input bytes halve.  Weights stay
    bf16 (mixed-dtype matmul is legal on TRN2).
  - the whole per-core input (61.5 KiB/partition) is resident in SBUF: one
    tile per super-block, never reused, so input DMA is gated only by the
    ring.  All input DMA rides the sync/SP HWDGE ring; one ring saturates
    HBM fine and the ACT queue stays clean for compute.
  - PSUM is rotated per 2-bank CHUNK (bufs=4, 4 chunks in flight) instead
    of per 4-bank super-block: the psum-free -> preload -> matmul -> tail
    loop is the critical cycle, and 4-deep rotation hides the cross-engine
    latency that paced v3/v4 (~2.7us/sb cadence vs ~2.4us busy).
  - tail per chunk: ACT squares the x-half (psum cols are (xp,g,k), so the
    x-half is one 3-dim strided slice) -> bf16; DVE casts the p-half to
    bf16 SBUF (DVE may read PSUM only once per op), squares it there at 2x
    (tensor_mul), pair-adds.  mult/add are emitted one chunk late so the
    DVE queue never stalls on its own chain.  ln1p runs once per
    super-block (960 cols amortizes the ACT op overhead) and is emitted
    one super-block late, after the psum readers, with the output DMA on
    gpsimd (SWDGE).
  - d-preload matmuls (ones @ dpat, j-matmuls accumulate on top) kept from
    the baseline: folding d into a shift of x is impossible (S[rows,:64]
    is exactly rank-deficient, residual 3e-2).
"""

import ml_dtypes
import numpy as np

import concourse.bacc as bacc
import concourse.mybir as mybir
import concourse.tile as tile
from concourse.bass_utils import run_bass_kernel_spmd

N = 64          # wires
OUT = 10        # measured wires / classes
NCORES = 8
JBLK = 48                  # j-blocks per full super-block
WIDTHS = [12] + [JBLK] * 9 + [36, 12]
JTOT = sum(WIDTHS)         # 492
R = 256 * JTOT             # per-core rows = 125952
CC = 128 * JTOT            # per-core xstack cols = 62976
B_PAD = R * NCORES         # 1007616
F32 = mybir.dt.float32
BF16 = mybir.dt.bfloat16
FP8 = mybir.dt.float8e3
NPBF16 = ml_dtypes.bfloat16
NPFP8 = ml_dtypes.float8_e3m4


# ---------------------------------------------------------------- host math
def _bs_pass(n, start, int_params):
    i = np.arange(start, n - 1, 2)
    j = i + 1
    theta = int_params[3 * i]
    phi = int_params[3 * i + 1]
    ct, st = np.cos(theta), np.sin(theta)
    cp, sp = np.cos(phi), np.sin(phi)
    S = np.eye(2 * n)
    S[i, i] = ct
    S[i, j] = -cp * st
    S[i, n + j] = -sp * st
    S[j, i] = cp * st
    S[j, j] = ct
    S[j, n + i] = -sp * st
    S[n + i, j] = sp * st
    S[n + i, n + i] = ct
    S[n + i, n + j] = -cp * st
    S[n + j, i] = sp * st
    S[n + j, n + i] = cp * st
    S[n + j, n + j] = ct
    return S


def _layer_symplectic(n, int1, squeezes, int2):
    M = _bs_pass(n, 0, int1)
    M = _bs_pass(n, 1, int1) @ M
    c = np.concatenate([np.cos(int1[2::3]), np.ones(1)])
    s = np.concatenate([np.sin(int1[2::3]), np.zeros(1)])
    Rm = np.block([[np.diag(c), np.diag(-s)], [np.diag(s), np.diag(c)]])
    Sq = np.diag(np.concatenate([np.exp(-squeezes), np.exp(squeezes)]))
    M = Sq @ (Rm @ M)
    M = _bs_pass(n, 0, int2) @ M
    M = _bs_pass(n, 1, int2) @ M
    return M


def _affine_map(layers):
    n = N
    S = np.eye(2 * n)
    d = np.zeros(2 * n)
    for int1, sq, int2, disp in layers:
        M = _layer_symplectic(n, int1, sq, int2)
        S = M @ S
        d = M @ d
        d[:n] += 2.0 * disp
    return S, d


def _device_constants(layers):
    S, d = _affine_map(layers)
    w = np.arange(OUT)
    rows = np.concatenate([w, N + w])
    cov = S @ S.T
    cov_term = cov[w, w] + cov[N + w, N + w]            # (10,)
    W2 = S[rows, :N].T                                  # (64, 20): (xp,k)
    d20 = d[rows] / 2.0                                 # (20,)
    covc = np.maximum(cov_term / 4.0 - 0.5, 0.0)

    # ln1p(mx^2+mp^2+covc) = ln1p((mx^2+mp^2)*a) + ln1p(covc), a=1/(1+covc)
    r20 = np.tile(np.sqrt(1.0 / (1.0 + covc)), 2)       # (20,)
    beta = np.log1p(covc).astype(np.float32)            # (10,)
    Wp = (W2 * r20).astype(np.float32)                  # (64, 20)
    dp = (d20 * r20).astype(np.float32)                 # (20,)

    # wcat column order (xp, g, k): col = xp*20 + g*10 + k
    Wh = Wp.astype(NPBF16)
    wcat = np.zeros((128, 40), NPBF16)
    for xp in range(2):
        wcat[0:64, xp * 20 + 0:xp * 20 + 10] = Wh[:, xp * 10:xp * 10 + 10]
        wcat[64:128, xp * 20 + 10:xp * 20 + 20] = Wh[:, xp * 10:xp * 10 + 10]

    # d pattern in matching order: d40[xp*20 + g*10 + k] = dp[xp*10+k]
    d40 = np.empty(40, np.float64)
    for xp in range(2):
        for g in range(2):
            d40[xp * 20 + g * 10:xp * 20 + g * 10 + 10] = (
                dp[xp * 10:xp * 10 + 10])
    ones = np.ones((128, 128), NPBF16)
    dpat = np.ascontiguousarray(np.broadcast_to(
        np.tile(d40, 12) / 128.0, (128, 480))).astype(NPBF16)
    return wcat, ones, dpat, beta, Wp, dp


# ---------------------------------------------------------------- bass build
def build_nc(widths=None):
    widths = widths or WIDTHS
    jtot = sum(widths)
    nsb = len(widths)
    nc = bacc.Bacc("TRN2", target_bir_lowering=False)
    xs = nc.dram_tensor("xs", (128, 128 * jtot), FP8, kind="ExternalInput")
    wst = nc.dram_tensor("wcat", (128, 40), BF16, kind="ExternalInput")
    onest = nc.dram_tensor("ones", (128, 128), BF16, kind="ExternalInput")
    dpatt = nc.dram_tensor("dpat", (128, 480), BF16, kind="ExternalInput")
    out = nc.dram_tensor("out", (128, 20 * jtot), BF16, kind="ExternalOutput")

    Square = mybir.ActivationFunctionType.Square
    Ln = mybir.ActivationFunctionType.Ln

    with tile.TileContext(nc) as tc:
        with (
            tc.tile_pool(name="const", bufs=1) as cpool,
            tc.tile_pool(name="xin", bufs=1) as xpool,
            tc.tile_pool(name="mid", bufs=4) as mpool,
            tc.tile_pool(name="ob", bufs=6) as opool,
            tc.tile_pool(name="ps", bufs=4, space="PSUM") as pspool,
        ):
            w_t = cpool.tile([128, 40], BF16)
            ones_t = cpool.tile([128, 128], BF16)
            dpat_t = cpool.tile([128, 480], BF16)

            # one resident tile per super-block; never reused, so the DMA
            # stream is gated only by the ring itself
            xtiles = []
            for i, wdt in enumerate(widths):
                xtiles.append(xpool.tile([128, 128 * wdt], FP8,
                                         tag=f"x{i}", name=f"x{i}"))

            sb_cols = []
            c0 = 0
            for wdt in widths:
                sb_cols.append(c0)
                c0 += 128 * wdt

            def issue_input(i):
                nc.sync.dma_start(xtiles[i][:], xs[:, sb_cols[i]:
                                                  sb_cols[i] + 128 * widths[i]])
                if i == 0:
                    nc.sync.dma_start(w_t[:], wst[:])
                    nc.sync.dma_start(ones_t[:], onest[:])
                    nc.sync.dma_start(dpat_t[:], dpatt[:])

            pend_sb = []         # (cp_sb, sqx_sb, v)
            pend_ln = []         # (v, oc, out_col)

            def drain_sb(keep):
                # one 960-wide mult + one 960-wide add per super-block
                # (vs per-chunk ops): fewer DVE instructions and sems on
                # the pacing engine.  Runs one super-block late so DVE
                # never waits on its own casts.
                while len(pend_sb) > keep:
                    cp_, sqx, v = pend_sb.pop(0)
                    sqp = mpool.tile([128, cp_.shape[1]], BF16, tag="sqp",
                                     name="sqp")
                    nc.vector.tensor_mul(sqp[:], cp_[:], cp_[:])
                    nc.vector.tensor_add(v[:], sqx[:], sqp[:])

            def drain_ln(keep):
                # ln for super-block N runs during N+1 (its input v is
                # long since ready); output DMA via SWDGE keeps the ACT
                # queue free
                while len(pend_ln) > keep:
                    v, oc, ob = pend_ln.pop(0)
                    o = opool.tile([128, oc], BF16, tag="o")
                    nc.scalar.activation(o[:], v[:], Ln, bias=1.0)
                    nc.gpsimd.dma_start(out[:, ob:ob + oc], o[:])

            PREFETCH = 3

            def emit_sb(i, jblk, v, voff):
                oc, nbank = 20 * jblk, jblk // 12
                tin = xtiles[i]
                nxt = i + PREFETCH
                if nxt < nsb:
                    issue_input(nxt)

                sqx_sb = mpool.tile([128, oc], BF16, tag="sqx")
                cp_sb = mpool.tile([128, oc], BF16, tag="cp")
                t0 = 0
                while t0 < nbank:
                    cb = min(2, nbank - t0)
                    psc = pspool.tile([128, cb, 512], F32, tag="ps",
                                      name="psc")
                    for t in range(cb):
                        nc.tensor.matmul(psc[:, t, 0:480], ones_t[:],
                                         dpat_t[:], start=True, stop=False)
                    for j in range(12 * cb):
                        jj = 12 * t0 + j
                        nc.tensor.matmul(
                            psc[:, j // 12, 40 * (j % 12):40 * (j % 12) + 40],
                            tin[:, 128 * jj:128 * jj + 128], w_t[:],
                            start=False, stop=True,
                        )
                    # two parallel psum readers per chunk: ACT squares the
                    # x-half, DVE casts the p-half to bf16 (DVE may read
                    # PSUM only once per op; the square happens later via
                    # tensor_mul at 2x).  Parallel readers free the psum
                    # tile faster than one big reader op would.
                    pv = psc[:, 0:cb, 0:480].rearrange(
                        "p t (j x c) -> p t j x c", x=2, c=20)
                    cw = cb * 240
                    sxv = sqx_sb[:, t0 * 240:t0 * 240 + cw].rearrange(
                        "p (t j c) -> p t j c", t=cb, c=20)
                    cpv = cp_sb[:, t0 * 240:t0 * 240 + cw].rearrange(
                        "p (t j c) -> p t j c", t=cb, c=20)
                    nc.scalar.activation(sxv, pv[:, :, :, 0, :], Square)
                    nc.vector.tensor_copy(cpv, pv[:, :, :, 1, :])
                    t0 += cb

                pend_sb.append((cp_sb, sqx_sb, v[:, voff:voff + oc]))
                drain_sb(keep=1)

            for i in range(min(PREFETCH, nsb)):
                issue_input(i)
            # ln + output DMA are batched over PAIRS of super-blocks: one
            # 1920-wide Ln amortizes the ACT op overhead, one output DMA
            # halves the SWDGE issue count.  Output columns of a pair are
            # contiguous, so the dram store is a single slice.
            i = 0
            while i < nsb:
                w0 = widths[i]
                w1 = widths[i + 1] if i + 1 < nsb else 0
                poc = 20 * (w0 + w1)
                v = mpool.tile([128, poc], BF16, tag="v", name="v")
                emit_sb(i, w0, v, 0)
                if w1:
                    emit_sb(i + 1, w1, v, 20 * w0)
                pend_ln.append((v, poc, (sb_cols[i] // 128) * 20))
                drain_ln(keep=1)
                i += 2
            drain_sb(keep=0)
            drain_ln(keep=0)
    nc.compile()
    return nc


# ---------------------------------------------------------------- host glue
def _make_in_maps(x_batch, wcat, ones, dpat):
    B = x_batch.shape[0]
    xpad = np.zeros((B_PAD, N), np.float32)
    xpad[:B] = x_batch
    xh = xpad.astype(NPFP8)
    in_maps = []
    for c in range(NCORES):
        xc = xh[c * R:(c + 1) * R]
        xstk = np.empty((128, CC), NPFP8)
        # per sb: rows (grp, t, f) -> xstk[grp*64+f, c0+t]
        r0 = c0 = 0
        for w in WIDTHS:
            half = 128 * w
            xt = xc[r0:r0 + 2 * half].reshape(2, half, N)
            xstk[:, c0:c0 + half] = xt.transpose(0, 2, 1).reshape(128, half)
            r0 += 2 * half
            c0 += half
        in_maps.append({"xs": xstk, "wcat": wcat, "ones": ones,
                        "dpat": dpat})
    return in_maps


def _decode_out(results, B):
    """Assemble the per-core bf16 outputs into the full (B, OUT) array."""
    full = np.empty((B_PAD, OUT), np.float32)
    for c in range(NCORES):
        O = results[c]["out"].astype(np.float32)
        r0 = o0 = 0
        for w in WIDTHS:
            Ow = O[:, o0:o0 + 20 * w].reshape(128, w, 2, OUT)
            full[c * R + r0:c * R + r0 + 256 * w] = (
                Ow.transpose(2, 1, 0, 3).reshape(256 * w, OUT))
            r0 += 256 * w
            o0 += 20 * w
    return full[:B]


_NC_CACHE = {}


def kernel(x_batch, int1_0, squeezes_0, int2_0, disp_0,
           int1_1, squeezes_1, int2_1, disp_1, _trace=False):
    layers = [
        (np.asarray(int1_0, np.float64), np.asarray(squeezes_0, np.float64),
         np.asarray(int2_0, np.float64), np.asarray(disp_0, np.float64)),
        (np.asarray(int1_1, np.float64), np.asarray(squeezes_1, np.float64),
         np.asarray(int2_1, np.float64), np.asarray(disp_1, np.float64)),
    ]
    wcat, ones, dpat, beta, Wp, dp = _device_constants(layers)
    xb = np.asarray(x_batch, np.float32)
    in_maps = _make_in_maps(xb, wcat, ones, dpat)

    if "nc" not in _NC_CACHE:
        _NC_CACHE["nc"] = build_nc()
    nc = _NC_CACHE["nc"]

    res = run_bass_kernel_spmd(
        nc, in_maps, core_ids=list(range(NCORES)), trace=_trace
    )
    out = _decode_out(res.results, x_batch.shape[0]) + beta
    if _trace:
        return out, res
    return out
